# revision 4
# baseline (speedup 1.0000x reference)
"""VQ codebook-lookup kernel for Trainium2 (8 NeuronCores, data-parallel).

Computes: idx = argmax(x, axis=-1); out = W.T[idx]  (i.e. out[n] = W[:, idx[n]])
  x: [8192, 8192] f32, W: [1024, 8192] f32 -> out: [8192, 1024] f32

Sharding: x split along tokens into 8 shards of [1024, 8192]; W.T replicated.
Each core: per-row argmax via VectorE max/max_index, then a DMA row-gather
from the transposed codebook in HBM.
"""

import sys

import numpy as np

sys.path.insert(0, "/opt/trn_rl_repo")

import concourse.bass as bass  # noqa: E402
import concourse.tile as tile  # noqa: E402
from concourse import bacc, bass_utils, mybir  # noqa: E402

N_CORES = 8
N_TOKENS = 8192
QUANT_DIM = 8192
OUT_DIM = 1024
ROWS = N_TOKENS // N_CORES  # rows (tokens) per core
P = 128  # SBUF partitions
N_TILES = ROWS // P  # row-tiles per core

F32 = mybir.dt.float32
I16 = mybir.dt.int16
I32 = mybir.dt.int32
U32 = mybir.dt.uint32

CH = 256  # chunk size for the hierarchical argmax
NCH = QUANT_DIM // CH  # 32 chunks per row

VERSION = 2


def _emit_kernel(tc: tile.TileContext, y: "bass.AP", x: "bass.AP", wt: "bass.AP"):
    """Per-core program. x: [ROWS, QUANT_DIM], wt: [QUANT_DIM, OUT_DIM] (=W.T),
    y: [ROWS, OUT_DIM]."""
    nc = tc.nc
    with (
        tc.tile_pool(name="xp", bufs=3) as xp,
        tc.tile_pool(name="sm", bufs=2 * N_TILES) as sm,
        tc.tile_pool(name="ip", bufs=1) as ip,
        tc.tile_pool(name="op", bufs=1) as op,
        tc.tile_pool(name="dr", bufs=1, space="DRAM") as dr,
    ):
        # Per-row argmax, one [128, QUANT_DIM] tile at a time.
        idx_all = ip.tile([P, N_TILES], I16)  # [row-in-tile, tile] argmax
        for t in range(N_TILES):
            xt = xp.tile([P, QUANT_DIM], F32)
            nc.sync.dma_start(xt[:], x[t * P : (t + 1) * P, :])
            mx = sm.tile([P, 8], F32, tag="mx")
            nc.vector.max(mx[:], xt[:])
            ix = sm.tile([P, 8], U32, tag="ix")
            nc.vector.max_index(ix[:], mx[:], xt[:])
            # argmax = ix[:, 0]; value < 8192 so the low half-word holds it.
            nc.vector.tensor_copy(idx_all[:, t : t + 1], ix.bitcast(I16)[:, 0:1])

        # dma_gather wants indices int16, "wrapped": gather j reads the index
        # at partition j%16, slot j//16 (replicated across the 8 gpsimd cores'
        # 16-partition groups). Row j = t*128 + p with p = s1*16 + q, so the
        # value for (q, slot=t*8+s1) is idx_all[s1*16+q, t]. Partition-crossing
        # shuffle goes through a DRAM scratch roundtrip.
        scratch = dr.tile([P, N_TILES], I16)
        nc.sync.dma_start(scratch[:], idx_all[:])
        idxw = ip.tile([P, ROWS // 16], I16)
        src = scratch.rearrange("(s1 q) t -> q t s1", q=16)
        for r in range(P // 16):
            dst = idxw[16 * r : 16 * (r + 1), :].rearrange(
                "q (t s1) -> q t s1", s1=N_TILES
            )
            nc.sync.dma_start(dst, src)

        # Gather rows of W.T from HBM: wout[p, t, :] = wt[idx[t*128+p], :]
        wout = op.tile([P, N_TILES, OUT_DIM], F32)
        nc.gpsimd.dma_gather(
            wout[:],
            wt[:],
            idxw[:],
            num_idxs=ROWS,
            num_idxs_reg=ROWS,
            elem_size=OUT_DIM,
        )
        nc.sync.dma_start(y.rearrange("(t p) d -> p t d", p=P), wout[:])


def _wrap_1024(nc, idx_sbuf, scratch, idxw):
    """Turn idx_sbuf [128, 8] int16 (value for row t*128+p at [p, t]) into the
    dma_gather wrapped layout idxw [128, 64]: gather j reads partition j%16,
    slot j//16; replicated across the 8 gpsimd-core partition groups.
    Row j = t*128 + s1*16 + q -> idxw[q, t*8+s1] = idx_sbuf[s1*16+q, t]."""
    nc.sync.dma_start(scratch[:], idx_sbuf[:])
    src = scratch.rearrange("(s1 q) t -> q t s1", q=16)
    for r in range(P // 16):
        dst = idxw[16 * r : 16 * (r + 1), :].rearrange(
            "q (t s1) -> q t s1", s1=N_TILES
        )
        nc.sync.dma_start(dst, src)


def _emit_kernel_v2(tc: tile.TileContext, y: "bass.AP", x: "bass.AP", wt: "bass.AP"):
    """Hierarchical argmax: one full pass computes per-chunk maxes (CH=256),
    cheap top-8 picks the winning chunk, a small HBM gather re-reads only the
    winning 1KB chunk per row, and a second tiny max/max_index finds the
    offset within it. ~1.06 passes of DVE work instead of 2."""
    nc = tc.nc
    with (
        tc.tile_pool(name="xp", bufs=3) as xp,
        tc.tile_pool(name="mp", bufs=N_TILES) as mp,
        tc.tile_pool(name="sm", bufs=2 * N_TILES) as sm,
        tc.tile_pool(name="keep", bufs=1) as keep,
        tc.tile_pool(name="op", bufs=1) as op,
        tc.tile_pool(name="dr", bufs=1, space="DRAM") as dr,
    ):
        # per-partition offsets: p*32 (global chunk id) as f32
        iota32 = keep.tile([P, 1], F32)
        nc.gpsimd.iota(
            iota32[:],
            pattern=[[0, 1]],
            base=0,
            channel_multiplier=NCH,
            allow_small_or_imprecise_dtypes=True,
        )

        cr_all = keep.tile([P, N_TILES], F32)  # winning chunk within row
        cidx_all = keep.tile([P, N_TILES], I16)  # global chunk id for gather
        # Phase A: chunk maxes + winning chunk per row.
        for t in range(N_TILES):
            xt = xp.tile([P, QUANT_DIM], F32)
            nc.sync.dma_start(xt[:], x[t * P : (t + 1) * P, :])
            m = mp.tile([P, NCH], F32, tag="m")
            nc.vector.reduce_max(
                m[:], xt.rearrange("p (c e) -> p c e", e=CH), axis=mybir.AxisListType.X
            )
            mx8 = sm.tile([P, 8], F32, tag="mx8")
            nc.vector.max(mx8[:], m[:])
            ci8 = sm.tile([P, 8], U32, tag="ci8")
            nc.vector.max_index(ci8[:], mx8[:], m[:])
            nc.vector.tensor_copy(cr_all[:, t : t + 1], ci8[:, 0:1])  # u32->f32
            gci = sm.tile([P, 1], F32, tag="gci")
            # global chunk id = (cr + t*128*NCH) + p*NCH
            nc.vector.tensor_scalar(
                gci[:],
                cr_all[:, t : t + 1],
                float(t * P * NCH),
                iota32[:],
                op0=mybir.AluOpType.add,
                op1=mybir.AluOpType.add,
            )
            nc.vector.tensor_copy(cidx_all[:, t : t + 1], gci[:])  # f32->i16

        # Phase B: gather each row's winning chunk (1KB) from x in HBM.
        scr_c = dr.tile([P, N_TILES], I16, tag="scr_c")
        idxw_c = keep.tile([P, ROWS // 16], I16, tag="idxw_c")
        _wrap_1024(nc, cidx_all, scr_c, idxw_c)
        wch = keep.tile([P, N_TILES, CH], F32)
        nc.gpsimd.dma_gather(
            wch[:],
            x.rearrange("r (c e) -> (r c) e", e=CH),
            idxw_c[:],
            num_idxs=ROWS,
            num_idxs_reg=ROWS,
            elem_size=CH,
        )

        # Phase C: offset within the winning chunk; final row-argmax.
        idx_all = keep.tile([P, N_TILES], I16)
        for t in range(N_TILES):
            wmx8 = sm.tile([P, 8], F32, tag="wmx8")
            nc.vector.max(wmx8[:], wch[:, t, :])
            wix8 = sm.tile([P, 8], U32, tag="wix8")
            nc.vector.max_index(wix8[:], wmx8[:], wch[:, t, :])
            wif = sm.tile([P, 1], F32, tag="wif")
            nc.vector.tensor_copy(wif[:], wix8[:, 0:1])  # u32->f32
            fin = sm.tile([P, 1], F32, tag="fin")
            # final = cr*CH + wi
            nc.vector.tensor_scalar(
                fin[:],
                cr_all[:, t : t + 1],
                float(CH),
                wif[:],
                op0=mybir.AluOpType.mult,
                op1=mybir.AluOpType.add,
            )
            nc.vector.tensor_copy(idx_all[:, t : t + 1], fin[:])  # f32->i16

        # Phase D: gather rows of W.T and write out.
        scr_w = dr.tile([P, N_TILES], I16, tag="scr_w")
        idxw_w = keep.tile([P, ROWS // 16], I16, tag="idxw_w")
        _wrap_1024(nc, idx_all, scr_w, idxw_w)
        wout = op.tile([P, N_TILES, OUT_DIM], F32)
        nc.gpsimd.dma_gather(
            wout[:],
            wt[:],
            idxw_w[:],
            num_idxs=ROWS,
            num_idxs_reg=ROWS,
            elem_size=OUT_DIM,
        )
        nc.sync.dma_start(y.rearrange("(t p) d -> p t d", p=P), wout[:])


_CACHE: dict[str, object] = {}


def _build():
    if "nc" in _CACHE:
        return _CACHE["nc"]
    nc = bacc.Bacc(
        "TRN2", target_bir_lowering=False, debug=False, enable_asserts=True
    )
    x = nc.dram_tensor("x", [ROWS, QUANT_DIM], F32, kind="ExternalInput").ap()
    wt = nc.dram_tensor("wt", [QUANT_DIM, OUT_DIM], F32, kind="ExternalInput").ap()
    y = nc.dram_tensor("y", [ROWS, OUT_DIM], F32, kind="ExternalOutput").ap()
    emit = _emit_kernel_v2 if VERSION == 2 else _emit_kernel
    with tile.TileContext(nc) as tc:
        emit(tc, y, x, wt)
    nc.compile()
    _CACHE["nc"] = nc
    return nc


def kernel(x: np.ndarray, W: np.ndarray, **_unused) -> np.ndarray:
    assert x.shape == (N_TOKENS, QUANT_DIM) and W.shape == (OUT_DIM, QUANT_DIM)
    nc = _build()
    x = np.ascontiguousarray(x, dtype=np.float32)
    wt = np.ascontiguousarray(W.T.astype(np.float32, copy=False))
    in_maps = [
        {"x": x[i * ROWS : (i + 1) * ROWS], "wt": wt} for i in range(N_CORES)
    ]
    res = bass_utils.run_bass_kernel_spmd(nc, in_maps, core_ids=list(range(N_CORES)))
    return np.concatenate([res.results[i]["y"] for i in range(N_CORES)], axis=0)
